# revision 56
# baseline (speedup 1.0000x reference)
"""CPI-MPNN (molecule MPNN + protein CNN + FC head) Trainium2 kernel.

Self-contained: hardcodes all shapes. Shards the batch (128) across 8
NeuronCores (16 samples each), replicates the small weights.

Strategy (v4 = v3 + fp8 DoubleRow conv tower):
  - Protein conv tower in fp8e4 with DoubleRow matmuls: two conv taps
    share one matmul (virtual K=256). The moving operand's tap pair is
    an overlapping 3D access pattern (pair stride = 1 element), so no
    activation duplication is needed. Cuts conv PE cycles ~40%.
  - conv0 input host-packed as channel-pair-interleaved fp8 with all 3
    taps stacked on 75 partitions: one DoubleRow matmul per 500-col
    chunk.
  - Activations carry power-of-2 fp8 scales (x*256); the scales fold
    into the activation scale/bias constants and the fc0 protein rows.
  - MPNN stays bf16 (it dominates the output numerically).
  - Single-matmul interleave of conv (N=500) and MPNN (small) matmuls
    keeps the PE duty cycle above the HAM clock-gate threshold.
  - PE warm-up matmuls during the initial DMA wait.
  - DMA descriptors batched into a few large blobs.
"""

import os
import numpy as np
from contextlib import ExitStack

import concourse.bass as bass
import concourse.tile as tile
from concourse import bacc, mybir
from concourse.bass_utils import run_bass_kernel_spmd



F32 = mybir.dt.float32
BF16 = mybir.dt.bfloat16
FP8 = mybir.dt.float8e4
AF = mybir.ActivationFunctionType
ALU = mybir.AluOpType
DRSW = mybir.MatmulPerfMode.DoubleRowSwInterleave

H = 200
B, NA, NB = 128, 48, 96
L = 1000

NCORES = 8
M = B // NCORES          # samples per core (16)
SEG = 1006               # 3 + 1000 + 3 padded segment
PAD = 3
NCH = 500                # conv free-dim chunk (2 per sample)
GM = 4                   # molecules per DMA group

# fp8 scaling: activations and weights stored *256; every conv layer's
# psum is then true*65536, un-scaled by the next activation's
# scale=2^-8 (and the pool bias / fc0 protein rows for the last layer).
ACT_SC = 1.0 / 256.0
POOL_SC = 65536.0

# weight blob column layouts (all DoubleRow stationaries padded to the
# full 128 columns — the PE rejects DR matmuls with num_active_cols<128)
EBF = 776                # bf16 early blob: wi | fbc group 0 (per-core)
EF8 = 896                # fp8 early blob: w0i | w1p0 | w1p1 | w1s
WBLOB = 1792             # fp8 conv2 blob: w2pA*3 | w2sA | w2pB*3 | w2sB
BLOB1 = 400              # bf16: wha | whb
BLOB2B = 601             # bf16: wo1 | wo2 | wo3 | ones
BLOB3 = 1001             # bf16: fc0 a-d | fc1 a/b | fc2

_CACHE = {}


def _dr_rhs(ap2d, n):
    """[P, 2, n] moving-operand AP with pair stride 2: the two virtual
    rows read taps t and t+2. Pair stride 1 (overlapping 16-bit lane
    reads) hard-crashes the PE when other engines touch SBUF."""
    return bass.AP(ap2d.tensor, ap2d.offset,
                   [list(ap2d.ap[0]), [2, 2], [1, n]])


def _build_nc():
    nc = bacc.Bacc("TRN2", target_bir_lowering=False, debug=False)

    d_x0 = nc.dram_tensor("x0p", [76, M, 2 * SEG], FP8, kind="ExternalInput")
    d_fb = nc.dram_tensor("fbc", [50, M, 144], BF16, kind="ExternalInput")
    d_ab = nc.dram_tensor("aba", [96, M, 144], BF16, kind="ExternalInput")
    d_ebf = nc.dram_tensor("earlybf", [50, EBF], BF16, kind="ExternalInput")
    d_ef8 = nc.dram_tensor("earlyf8", [97, EF8], FP8, kind="ExternalInput")
    d_wf8 = nc.dram_tensor("wf8", [128, WBLOB], FP8, kind="ExternalInput")
    d_w1b = nc.dram_tensor("wblob1", [128, BLOB1], BF16, kind="ExternalInput")
    d_w2bb = nc.dram_tensor("wblob2b", [128, BLOB2B], BF16,
                            kind="ExternalInput")
    d_w3b = nc.dram_tensor("wblob3", [128, BLOB3], BF16, kind="ExternalInput")
    d_bias = nc.dram_tensor("biases", [128, 8], F32, kind="ExternalInput")
    d_out = nc.dram_tensor("out", [1, M], F32, kind="ExternalOutput")

    with tile.TileContext(nc) as tc, ExitStack() as ctx:
        cst = ctx.enter_context(tc.tile_pool(name="cst", bufs=1))
        sbs = ctx.enter_context(tc.tile_pool(name="sbs", bufs=1))
        tmp = ctx.enter_context(tc.tile_pool(name="tmp", bufs=1))
        xp = ctx.enter_context(tc.tile_pool(name="xp", bufs=1))
        pp = ctx.enter_context(tc.tile_pool(name="pp", bufs=1, space="PSUM"))

        # ---- PE warm-up (HAM clock ramp during the DMA wait) ----
        # memset on the vector engine: ~0.3us vs ~0.7us on gpsimd, and
        # DVE's queue comes up early, so the PE starts sooner.
        warm = cst.tile([128, 628], BF16, tag="warm")
        nc.vector.memset(warm[:], 0.0)
        warm_ps = pp.tile([128, NCH], F32, tag="cv", bufs=3, name="warm_ps")

        def emit_warm():
            nc.tensor.matmul(warm_ps[:], warm[:, 0:128], warm[:, 128:628],
                             start=True, stop=True)

        for _ in range(3):
            emit_warm()

        # ---- input DMAs (batched descriptors) ----
        x0_bufs = {}

        def x0_dma(s0, n, eng):
            t = xp.tile([76, n * 2 * SEG], FP8, tag=f"x0s{s0}",
                        name=f"x0s{s0}")
            eng.dma_start(t[:].rearrange("p (s c) -> p s c", c=2 * SEG),
                          d_x0.ap()[:, s0:s0 + n, :])
            for i in range(n):
                x0_bufs[s0 + i] = (t, i * 2 * SEG)

        def x0_stream(s):
            # called from gen_conv0(s): fetch the pair two samples ahead
            nx = s + 2
            if nx % 2 == 0 and nx < M and nx not in x0_bufs:
                eng = nc.sync if nx == 2 else (
                    nc.scalar if (nx // 2) % 2 == 0 else nc.sync)
                x0_dma(nx, 2, eng)

        fb_g, ab_g = {}, {}

        def grp_dma(g, eng, fb_too=True):
            t = cst.tile([96, GM * 144], BF16, tag=f"ab{g}")
            eng.dma_start(t[:].rearrange("p (m i) -> p m i", i=144),
                          d_ab.ap()[:, GM * g:GM * (g + 1), :])
            ab_g[g] = t
            if fb_too:
                t = cst.tile([50, GM * 144], BF16, tag=f"fb{g}")
                eng.dma_start(t[:].rearrange("p (m i) -> p m i", i=144),
                              d_fb.ap()[:, GM * g:GM * (g + 1), :])
                fb_g[g] = t

        # ACT queue (its DMA queue comes up several us before sync's):
        # everything the conv stream needs, in need order, plus the
        # mol stream's critical first inputs packed into earlybf.
        ebf = cst.tile([50, EBF], BF16, tag="ebf")
        nc.scalar.dma_start(ebf[:], d_ebf.ap())
        fb_g[0] = ebf[0:50, 200:EBF]
        ef8 = cst.tile([97, EF8], FP8, tag="ef8")
        nc.scalar.dma_start(ef8[:], d_ef8.ap())
        x0_dma(0, 1, nc.scalar)
        wf8 = cst.tile([128, WBLOB], FP8, tag="wf8")
        nc.scalar.dma_start(wf8[:], d_wf8.ap())
        x0_dma(1, 1, nc.scalar)
        # SP queue: the rest in landing order.
        grp_dma(0, nc.sync, fb_too=False)
        biases = cst.tile([128, 8], F32, tag="biases")
        nc.sync.dma_start(biases[:], d_bias.ap())
        blob1 = cst.tile([128, BLOB1], BF16, tag="blob1")
        nc.sync.dma_start(blob1[:], d_w1b.ap())
        blob2b = cst.tile([128, BLOB2B], BF16, tag="blob2b")
        nc.sync.dma_start(blob2b[:], d_w2bb.ap())
        grp_dma(1, nc.sync)
        grp_dma(2, nc.sync)
        grp_dma(3, nc.sync)
        blob3 = cst.tile([128, BLOB3], BF16, tag="blob3")
        nc.sync.dma_start(blob3[:], d_w3b.ap())

        # fp8 conv weight views: SW-interleaved pair blocks (contiguous
        # fast weight load; plain DoubleRow's HW interleave pays ~150ns
        # extra LDWEIGHTS whenever a short matmul precedes it). conv0/
        # conv1 carry their bias as an extra contraction row (b*512
        # against a constant-128 activation row) so the relu runs as a
        # two-op tensor_scalar on the otherwise-idle gpsimd engine.
        w0i_t = ef8[0:76, 0:256]
        w1p_t = [ef8[0:96, 256 + 256 * j:512 + 256 * j] for j in range(2)]
        w1s_t = ef8[0:97, 768:896]
        w2pA_t = [wf8[0:128, 256 * j:256 + 256 * j] for j in range(3)]
        w2sA_t = wf8[0:128, 768:896]
        w2pB_t = [wf8[0:128, 896 + 256 * j:1152 + 256 * j] for j in range(3)]
        w2sB_t = wf8[0:128, 1664:1792]

        # bf16 weight views
        wi_t = ebf[0:50, 0:200]
        wha_t = blob1[0:128, 0:200]
        whb_t = blob1[0:72, 200:400]
        wo1_t = blob2b[0:40, 0:200]
        wo2_t = blob2b[0:128, 200:400]
        wo3_t = blob2b[0:72, 400:600]
        fc0_t = [blob3[0:128, 0:200], blob3[0:72, 200:400],
                 blob3[0:128, 400:600], blob3[0:72, 600:800]]
        fc1a_t = blob3[0:128, 800:900]
        fc1b_t = blob3[0:72, 900:1000]
        fc2w_t = blob3[0:100, 1000:1001]
        b0_t = biases[0:96, 0:1]
        b1_t = biases[0:128, 1:2]
        b2a_t = biases[0:128, 2:3]
        b2b_t = biases[0:72, 3:4]
        fc0ba_t = biases[0:128, 4:5]
        fc0bb_t = biases[0:72, 5:6]
        fc1bias_t = biases[0:100, 6:7]
        fc2b_t = biases[0:1, 7:8]

        # static outputs of the two towers, feature-major [feat, M]
        embT1 = sbs.tile([128, M], BF16, tag="embT1")
        embT2 = sbs.tile([72, M], BF16, tag="embT2")
        embT1f = sbs.tile([128, M], F32, tag="embT1f")
        embT2f = sbs.tile([72, M], F32, tag="embT2f")
        prT1p = sbs.tile([128, M], F32, tag="prT1p")
        prT2p = sbs.tile([72, M], F32, tag="prT2p")

        # ================= per-molecule MPNN (staged generators) =======
        mol_state = {}

        def gen_binput(m):
            g, r = m // GM, m % GM
            fb_m = fb_g[g][:, r * 144:r * 144 + 96]
            ps = pp.tile([96, 200], F32, tag="mp", bufs=3)
            nc.tensor.matmul(ps[:], fb_m, wi_t, start=True, stop=True)
            binp = sbs.tile([96, 200], F32, tag=f"binp{m}")
            nc.vector.tensor_copy(binp[:], ps[:])
            msg = sbs.tile([96, 200], BF16, tag=f"msg{m}")
            nc.scalar.activation(msg[:], ps[:], AF.Relu)
            mol_state[m] = (binp, msg)
            yield

        def gen_iter_pre(m):
            g, r = m // GM, m % GM
            ab_m = ab_g[g][:, r * 144:r * 144 + 96]
            binp, msg = mol_state[m]
            # pa/pb share one PSUM bank slot; both are single-matmul
            # accumulation groups so the bank-granular pending-zero mark
            # of the second can't corrupt the first mid-group.
            nt = pp.tile([128, 192], F32, tag="nt", bufs=2)
            nc.tensor.matmul(nt[0:128, 0:96], msg[:, 0:128], ab_m,
                             start=True, stop=True)
            yield
            nc.tensor.matmul(nt[0:72, 96:192], msg[:, 128:200], ab_m,
                             start=True, stop=True)
            nTa = tmp.tile([128, 96], BF16, tag="nTa", bufs=6)
            nc.vector.tensor_copy(nTa[:], nt[0:128, 0:96])
            nTb = tmp.tile([72, 96], BF16, tag="nTb", bufs=6)
            nc.vector.tensor_copy(nTb[:], nt[0:72, 96:192])
            mol_state[m] = (binp, msg, nTa, nTb)
            yield

        def gen_iter_post(m):
            binp, msg, nTa, nTb = mol_state[m]
            ps = pp.tile([96, 200], F32, tag="mp", bufs=3)
            nc.tensor.matmul(ps[:], nTa[:], wha_t, start=True, stop=False)
            yield
            nc.tensor.matmul(ps[:], nTb[:], whb_t, start=False, stop=True)
            tm = tmp.tile([96, 200], F32, tag="mtmp", bufs=3)
            nc.vector.tensor_add(tm[:], ps[:], binp[:])
            nc.scalar.activation(msg[:], tm[:], AF.Relu)
            mol_state[m] = (binp, msg)
            yield

        atom_g = {}

        def gen_atom_pre(m):
            g, r = m // GM, m % GM
            aa_m = ab_g[g][:, r * 144 + 96:(r + 1) * 144]
            binp, msg = mol_state[m]
            if r == 0:
                atom_g[g] = (tmp.tile([128, 192], BF16, tag="nat1", bufs=2,
                                      name="nat1g"),
                             tmp.tile([72, 192], BF16, tag="nat2", bufs=2,
                                      name="nat2g"))
            nat1_g, nat2_g = atom_g[g]
            pT = pp.tile([128, 96], F32, tag="nt", bufs=2)
            nc.tensor.matmul(pT[0:128, 0:48], msg[:, 0:128], aa_m,
                             start=True, stop=True)
            yield
            nc.tensor.matmul(pT[0:72, 48:96], msg[:, 128:200], aa_m,
                             start=True, stop=True)
            nc.scalar.copy(nat1_g[:, r * 48:(r + 1) * 48], pT[0:128, 0:48])
            nc.scalar.copy(nat2_g[:, r * 48:(r + 1) * 48], pT[0:72, 48:96])
            yield

        def gen_atom_mm(g):
            # batched over the 4 molecules of the group: one stationary
            # load per W_o block, moving operand N=192. Atom hidden
            # states come out H-major so the atom mean collapses into
            # the activation's accum_out; W_o pre-scaled 1/48 on host.
            nat1_g, nat2_g = atom_g[g]
            fb0 = fb_g[g][0:40, 96:97]
            c1_g = bass.AP(fb0.tensor, fb0.offset,
                           [list(fb0.ap[0]), [144, GM], [1, 48]])
            for half, wid, embf in ((0, 128, embT1f), (1, 72, embT2f)):
                lo, hi = (0, 128) if half == 0 else (128, 200)
                ps = pp.tile([wid, 192], F32, tag="nt", bufs=2,
                             name=f"psAH{half}")
                nc.tensor.matmul(ps[:], wo1_t[:, lo:hi], c1_g,
                                 start=True, stop=False)
                yield
                nc.tensor.matmul(ps[:], wo2_t[:, lo:hi], nat1_g[:],
                                 start=False, stop=False)
                yield
                nc.tensor.matmul(ps[:], wo3_t[:, lo:hi], nat2_g[:],
                                 start=False, stop=True)
                for r in range(GM):
                    rh = tmp.tile([wid, 48], BF16, tag="reluh", bufs=4,
                                  name=f"r{half}")
                    nc.scalar.activation(rh[:], ps[:, r * 48:(r + 1) * 48],
                                         AF.Relu,
                                         accum_out=embf[:, GM * g + r:
                                                        GM * g + r + 1])
                yield

        # ================= per-sample protein conv tower (fp8) =========
        sample_state = {}

        def gen_conv0(s):
            x0t, off = x0_bufs[s]
            x1 = xp.tile([97, SEG], FP8, tag="x1", bufs=3)
            nc.gpsimd.memset(x1[0:96, 0:PAD], 0.0)
            nc.gpsimd.memset(x1[0:96, PAD + 1000:SEG], 0.0)
            nc.gpsimd.memset(x1[96:97, :], 128.0)   # conv1 bias-row input
            for c in range(2):
                base = off + PAD + c * NCH
                ps = pp.tile([128, NCH], F32, tag="cv", bufs=3)
                # host layout: [76, 2, SEG] = 3 taps x 25 ch-pairs (+ a
                # constant bias row), pre-shifted per tap; one DoubleRow
                # matmul per chunk.
                rhs = bass.AP(x0t[:].tensor, x0t[:, base:base + 1].offset,
                              [list(x0t[:].ap[0]), [SEG, 2], [1, NCH]])
                nc.tensor.matmul(ps[:], w0i_t, rhs, start=True, stop=True,
                                 perf_mode=DRSW)
                yield
                nc.vector.tensor_scalar(
                    x1[0:96, PAD + c * NCH:PAD + (c + 1) * NCH], ps[0:96, :],
                    ACT_SC, 0.0, op0=ALU.mult, op1=ALU.max)
                yield
            x0_stream(s)
            sample_state[s] = [x1, None, None, None]

        # chunk splits chosen so chunk 0 of each layer reads only chunk
        # 0 of its producer (the conv window would otherwise reach 2-3
        # columns past the boundary and serialize on BOTH activations).
        C1SPL = (0, 498, 1000)
        C2SPL = (0, 495, 1000)

        def gen_conv1(s, c):
            st = sample_state[s]
            x1 = st[0]
            if c == 0:
                x2 = xp.tile([128, SEG], FP8, tag="x2", bufs=4)
                nc.gpsimd.memset(x2[:, 0:PAD], 0.0)
                nc.gpsimd.memset(x2[:, PAD + 1000:SEG], 0.0)
                st[1] = x2
            x2 = st[1]
            base = PAD + C1SPL[c]
            n = C1SPL[c + 1] - C1SPL[c]
            ps = pp.tile([128, n], F32, tag="cv", bufs=3, name="c1ps")
            for j, t in enumerate((0, 1)):       # pairs (0,2) and (1,3)
                rhs = _dr_rhs(x1[0:96, base + t - 2:base + t + n], n)
                nc.tensor.matmul(ps[:], w1p_t[j], rhs, start=(j == 0),
                                 stop=False, perf_mode=DRSW)
                yield
            nc.tensor.matmul(ps[:], w1s_t, x1[0:97, base + 2:base + 2 + n],
                             start=False, stop=True)
            nc.scalar.activation(x2[:, base:base + n], ps[:],
                                 AF.Relu, scale=ACT_SC)
            yield

        def gen_conv2(s, c, half):
            st = sample_state[s]
            x2 = st[1]
            if half == 0:
                if c == 0:
                    st[2] = tmp.tile([128, 2], F32, tag="mxA", bufs=3,
                                     name="mxA")
                    st[3] = tmp.tile([72, 2], F32, tag="mxB", bufs=3,
                                     name="mxB")
                wp, ws, use, mx = w2pA_t, w2sA_t, 128, st[2]
            else:
                wp, ws, use, mx = w2pB_t, w2sB_t, 72, st[3]
            base = PAD + C2SPL[c]
            n = C2SPL[c + 1] - C2SPL[c]
            ps = pp.tile([128, n], F32, tag="cv", bufs=3, name="c2ps")
            for j, t in enumerate((0, 4, 1)):    # pairs (0,2) (4,6) (1,3)
                rhs = _dr_rhs(x2[:, base + t - 3:base + t - 1 + n], n)
                nc.tensor.matmul(ps[:], wp[j], rhs, start=(j == 0),
                                 stop=False, perf_mode=DRSW)
                yield
            nc.tensor.matmul(ps[:], ws, x2[:, base + 2:base + 2 + n],
                             start=False, stop=True)
            nc.vector.reduce_max(mx[:, c:c + 1], ps[0:use, :],
                                 axis=mybir.AxisListType.X)
            if c == 1:
                dst = prT1p if half == 0 else prT2p
                nc.vector.reduce_max(dst[:, s:s + 1], mx[:],
                                     axis=mybir.AxisListType.X)
                if half == 1:
                    sample_state.pop(s)
            yield

        # FC first-layer matmuls on the molecule embeddings run woven
        # into the last conv pieces; psums allocated lazily to preserve
        # the nt slot rotation.
        fc_ps = {}

        def gen_fc_early():
            nc.gpsimd.tensor_copy(embT1[:], embT1f[:])
            nc.gpsimd.tensor_copy(embT2[:], embT2f[:])
            fc_ps["a"] = pp.tile([128, M], F32, tag="nt", bufs=2, name="ps0a")
            fc_ps["b"] = pp.tile([72, M], F32, tag="nt", bufs=2, name="ps0b")
            rhs2 = (embT1, embT2)
            for k in range(2):
                nc.tensor.matmul(fc_ps["a"][:], fc0_t[k][:, 0:128],
                                 rhs2[k][:], start=(k == 0), stop=False)
                yield
            for k in range(2):
                nc.tensor.matmul(fc_ps["b"][:], fc0_t[k][:, 128:200],
                                 rhs2[k][:], start=(k == 0), stop=False)
                yield

        # ---- build the two global piece streams and weave them ----
        conv_pieces = []
        mol_pieces = []
        for g in range(4):
            s0, s1, s2, s3 = (4 * g + i for i in range(4))
            mols = [GM * g + r for r in range(GM)]
            for stage in (gen_binput, gen_iter_pre, gen_iter_post,
                          gen_iter_pre, gen_iter_post, gen_atom_pre):
                for m in mols:
                    mol_pieces.append(stage(m))
            mol_pieces.append(gen_atom_mm(g))
            conv_pieces += [
                gen_conv0(s0), gen_conv0(s1),
                gen_conv1(s0, 0), gen_conv1(s1, 0),
                gen_conv1(s0, 1), gen_conv1(s1, 1),
                gen_conv2(s0, 0, 0), gen_conv2(s0, 0, 1),
                gen_conv2(s1, 0, 0), gen_conv2(s1, 0, 1),
                gen_conv0(s2), gen_conv1(s2, 0),
                gen_conv2(s0, 1, 0), gen_conv2(s0, 1, 1),
                gen_conv1(s2, 1),
                gen_conv2(s1, 1, 0), gen_conv2(s1, 1, 1),
                gen_conv0(s3), gen_conv1(s3, 0), gen_conv1(s3, 1),
                gen_conv2(s2, 0, 0), gen_conv2(s2, 0, 1),
                gen_conv2(s2, 1, 0), gen_conv2(s2, 1, 1),
                gen_conv2(s3, 0, 0), gen_conv2(s3, 0, 1),
                gen_conv2(s3, 1, 0), gen_conv2(s3, 1, 1),
            ]
        mol_pieces.append(gen_fc_early())

        def stream(pieces):
            for p in pieces:
                yield from p

        cs = stream(conv_pieces)
        ms = stream(mol_pieces)
        # lead with the first mol group's binput+iter_pre; warmup
        # matmuls fill this phase's dependency stalls
        for _ in range(12):
            next(ms, None)
            emit_warm()
        conv_alive = mol_alive = True
        credit = 8.0
        RATIO = 104.0 / 51.0
        while conv_alive or mol_alive:
            credit += RATIO
            while credit >= 1.0 and conv_alive:
                credit -= 1.0
                if next(cs, StopIteration) is StopIteration:
                    conv_alive = False
            if mol_alive and next(ms, StopIteration) is StopIteration:
                mol_alive = False
            if not conv_alive:
                credit = 0.0

        # maxpool -> bias -> relu (monotone, so pool-first is exact);
        # prT carries the 65536x conv2 scale, folded into fc0 rows.
        prT1 = sbs.tile([128, M], BF16, tag="prT1")
        nc.vector.tensor_scalar(prT1[:], prT1p[:], b2a_t, 0.0,
                                op0=ALU.add, op1=ALU.max)
        prT2 = sbs.tile([72, M], BF16, tag="prT2")
        nc.vector.tensor_scalar(prT2[:], prT2p[:], b2b_t, 0.0,
                                op0=ALU.add, op1=ALU.max)

        # ================= FC head (tail) =================
        rhs4 = (embT1, embT2, prT1, prT2)
        for k in (2, 3):
            nc.tensor.matmul(fc_ps["a"][:], fc0_t[k][:, 0:128], rhs4[k][:],
                             start=False, stop=(k == 3))
        h0a = tmp.tile([128, M], BF16, tag="h0a")
        nc.scalar.activation(h0a[:], fc_ps["a"][:], AF.Relu, bias=fc0ba_t)
        for k in (2, 3):
            nc.tensor.matmul(fc_ps["b"][:], fc0_t[k][:, 128:200], rhs4[k][:],
                             start=False, stop=(k == 3))
        h0b = tmp.tile([72, M], BF16, tag="h0b")
        nc.scalar.activation(h0b[:], fc_ps["b"][:], AF.Relu, bias=fc0bb_t)

        ps1 = pp.tile([100, M], F32, tag="nt", bufs=2)
        nc.tensor.matmul(ps1[:], fc1a_t, h0a[:], start=True, stop=False)
        nc.tensor.matmul(ps1[:], fc1b_t, h0b[:], start=False, stop=True)
        h1 = tmp.tile([100, M], BF16, tag="h1")
        nc.scalar.activation(h1[:], ps1[:], AF.Relu, bias=fc1bias_t)

        ps2 = pp.tile([1, M], F32, tag="nt", bufs=2)
        nc.tensor.matmul(ps2[:], fc2w_t, h1[:], start=True, stop=True)
        outsb = tmp.tile([1, M], F32, tag="outsb")
        nc.scalar.add(outsb[:], ps2[:], fc2b_t[0:1, 0:1])
        nc.sync.dma_start(d_out.ap(), outsb[:])

    nc.compile()
    return nc


def _prep(inputs):
    """Host preprocessing: returns the 8 per-core in_maps."""
    import ml_dtypes
    f32 = np.float32
    bf16 = ml_dtypes.bfloat16
    fp8 = ml_dtypes.float8_e4m3
    SC = 256.0
    fatoms = np.asarray(inputs["fatoms"], f32)
    fbonds = np.asarray(inputs["fbonds"], f32)
    agraph = np.asarray(inputs["agraph"])
    bgraph = np.asarray(inputs["bgraph"])
    pseq = np.asarray(inputs["protein_seq"])
    W_i = np.asarray(inputs["W_i"], f32)
    W_h = np.asarray(inputs["W_h"], f32)
    W_o_w = np.asarray(inputs["W_o_w"], f32)
    W_o_b = np.asarray(inputs["W_o_b"], f32)
    embp = np.asarray(inputs["embed_protein"], f32)

    # protein embeddings *256, channel-pair interleaved with the 3 taps
    # pre-shifted and stacked on 75 partitions (+ the conv0 bias input
    # row, constant 128): [76, 2, SEG] per sample
    pvT = np.ascontiguousarray(embp[pseq].transpose(0, 2, 1)) * SC  # (B,50,L)
    x0i = np.zeros((B, 76, 2, SEG), f32)
    for tap in range(3):
        sh = tap - 1                      # position shift: pv[ch, c-PAD+sh]
        lo, hi = max(0, PAD - sh), min(SEG, PAD + L - sh)
        blk = x0i[:, tap * 25:(tap + 1) * 25]
        blk[:, :, 0, lo:hi] = pvT[:, 0::2, lo - PAD + sh:hi - PAD + sh]
        blk[:, :, 1, lo:hi] = pvT[:, 1::2, lo - PAD + sh:hi - PAD + sh]
    x0i[:, 75, 0, :] = 128.0
    x0i = x0i.reshape(B, 76, 2 * SEG).astype(fp8)

    # adjacency one-hots (counts; contraction-dim-major)
    ar = np.arange(B)[:, None, None]
    cntB = np.zeros((B, NB, NB), f32)
    np.add.at(cntB, (ar, np.arange(NB)[None, :, None], bgraph), 1.0)
    abt = cntB.transpose(0, 2, 1)                              # (B, j, i)
    cntA = np.zeros((B, NA, NB), f32)
    np.add.at(cntA, (ar, np.arange(NA)[None, :, None], agraph), 1.0)
    aat = cntA.transpose(0, 2, 1)                              # (B, j, a)

    # combined group inputs: ab = [abt | aat] on 96 partitions,
    # fb = [fbonds.T | cat1 (40 rows + 10 pad)] on 50 partitions
    aba = np.concatenate([abt, aat], axis=2)                   # (B, 96, 144)
    fbT = fbonds.transpose(0, 2, 1)                            # (B, 50, 96)
    faT = fatoms.transpose(0, 2, 1)                            # (B, 39, 48)
    cat1 = np.concatenate(
        [faT, np.ones((B, 1, NA), f32), np.zeros((B, 10, NA), f32)], axis=1)
    fbc = np.concatenate([fbT, cat1], axis=2)                  # (B, 50, 144)

    conv_w = [np.asarray(inputs[f"conv{i}_w"], f32) for i in range(3)]
    conv_b = [np.asarray(inputs[f"conv{i}_b"], f32) for i in range(3)]
    fcw = [np.asarray(inputs[f"fc{i}_w"], f32) for i in range(3)]
    fcb = [np.asarray(inputs[f"fc{i}_b"], f32) for i in range(3)]

    def put(blob, col, rows, a):
        blob[0:rows, col:col + a.shape[1]] = a
        return col + a.shape[1]

    def sw_pair(dst, rows, col, wa, wb):
        """Store a DoubleRowSwInterleave pair block: viewed col 2k =
        A[:, 127-k], col 2k+1 = B[:, 127-k]; A/B zero-padded to the
        mandatory 128 columns."""
        a = np.zeros((rows, 128), f32)
        a[:, 0:wa.shape[1]] = wa
        b = np.zeros((rows, 128), f32)
        b[:, 0:wb.shape[1]] = wb
        dst[0:rows, col:col + 256:2] = a[:, ::-1]
        dst[0:rows, col + 1:col + 256:2] = b[:, ::-1]

    # fp8 early blob: w0i | w1p0 | w1p1 | w1s  (all *256; tap pairs are
    # (t, t+2) — the DR moving operand uses pair stride 2). The conv
    # biases ride as extra contraction rows: weight b*512 against the
    # constant-128 activation row gives b*65536 in the psum.
    ef8 = np.zeros((97, EF8), f32)
    w0 = conv_w[0] * SC                                        # (96, 50, 3)
    w0a = np.zeros((76, 96), f32)
    w0b = np.zeros((76, 96), f32)
    for tap in range(3):
        w0a[tap * 25:(tap + 1) * 25] = w0[:, 0::2, tap].T
        w0b[tap * 25:(tap + 1) * 25] = w0[:, 1::2, tap].T
    w0a[75] = conv_b[0] * 512.0
    sw_pair(ef8, 76, 0, w0a, w0b)
    w1 = conv_w[1] * SC                                        # (128, 96, 5)
    for j, t in enumerate((0, 1)):
        sw_pair(ef8, 96, 256 + 256 * j, w1[:, :, t].T, w1[:, :, t + 2].T)
    ef8[0:96, 768:896] = w1[:, :, 4].T
    ef8[96, 768:896] = conv_b[1] * 512.0
    # fp8 conv2 blob: w2pA*3 | w2sA | w2pB*3 | w2sB
    wf8 = np.zeros((128, WBLOB), f32)
    w2 = conv_w[2] * SC                                        # (200, 128, 7)
    for j, t in enumerate((0, 4, 1)):
        sw_pair(wf8, 128, 256 * j, w2[0:128, :, t].T, w2[0:128, :, t + 2].T)
        sw_pair(wf8, 128, 896 + 256 * j,
                w2[128:200, :, t].T, w2[128:200, :, t + 2].T)
    wf8[0:128, 768:896] = w2[0:128, :, 5].T
    wf8[0:128, 1664:1736] = w2[128:200, :, 5].T

    # blob1: wha | whb  (bf16 MPNN weights)
    blob1 = np.zeros((128, BLOB1), f32)
    c = put(blob1, 0, 128, W_h[0:128])
    c = put(blob1, c, 72, W_h[128:200])
    assert c == BLOB1

    # blob2b: wo1 | wo2 | wo3 | ones
    wo1 = np.zeros((40, 200), f32)
    wo1[:39] = W_o_w[0:39] / 48.0
    wo1[39] = W_o_b / 48.0
    blob2b = np.zeros((128, BLOB2B), f32)
    c = put(blob2b, 0, 40, wo1)
    c = put(blob2b, c, 128, W_o_w[39:167] / 48.0)
    c = put(blob2b, c, 72, W_o_w[167:239] / 48.0)
    c = put(blob2b, c, 48, np.ones((48, 1), f32))
    assert c == BLOB2B

    # blob3: fc0 a-d | fc1 a/b | fc2; the fc0 protein rows (200:400)
    # un-scale the 65536x pooled conv2 output.
    fc0 = fcw[0].copy()
    fc0[200:400] /= POOL_SC
    blob3 = np.zeros((128, BLOB3), f32)
    c = put(blob3, 0, 128, fc0[0:128])
    c = put(blob3, c, 72, fc0[128:200])
    c = put(blob3, c, 128, fc0[200:328])
    c = put(blob3, c, 72, fc0[328:400])
    c = put(blob3, c, 128, fcw[1][0:128])
    c = put(blob3, c, 72, fcw[1][128:200])
    c = put(blob3, c, 100, fcw[2])
    assert c == BLOB3

    bias = np.zeros((128, 8), f32)
    bias[0:96, 0] = conv_b[0] * SC
    bias[0:128, 1] = conv_b[1] * SC
    bias[0:128, 2] = conv_b[2][0:128] * POOL_SC
    bias[0:72, 3] = conv_b[2][128:200] * POOL_SC
    bias[0:128, 4] = fcb[0][0:128]
    bias[0:72, 5] = fcb[0][128:200]
    bias[0:100, 6] = fcb[1]
    bias[0:1, 7] = fcb[2]

    shared = {
        "earlyf8": ef8.astype(fp8),
        "wf8": wf8.astype(fp8),
        "wblob1": blob1.astype(bf16),
        "wblob2b": blob2b.astype(bf16),
        "wblob3": blob3.astype(bf16), "biases": bias,
    }

    x0_cm = x0i.transpose(1, 0, 2)                             # (75, B, 2SEG)
    fbc_cm = fbc.transpose(1, 0, 2)                            # (50, B, 144)
    aba_cm = aba.transpose(1, 0, 2)                            # (96, B, 144)

    in_maps = []
    for cix in range(NCORES):
        lo = cix * M
        im = dict(shared)
        im["x0p"] = np.ascontiguousarray(x0_cm[:, lo:lo + M, :])
        im["fbc"] = np.ascontiguousarray(fbc_cm[:, lo:lo + M, :]).astype(bf16)
        im["aba"] = np.ascontiguousarray(aba_cm[:, lo:lo + M, :]).astype(bf16)
        # earlybf: W_i | this core's group-0 fbc (molecules 0-3)
        ebf = np.zeros((50, EBF), f32)
        ebf[:, 0:200] = W_i
        ebf[:, 200:EBF] = fbc_cm[:, lo:lo + GM, :].reshape(50, GM * 144)
        im["earlybf"] = ebf.astype(bf16)
        in_maps.append(im)
    return in_maps


def get_nc():
    if "nc" not in _CACHE:
        _CACHE["nc"] = _build_nc()
    return _CACHE["nc"]


def kernel(**inputs) -> np.ndarray:
    nc = get_nc()
    in_maps = _prep(inputs)
    res = run_bass_kernel_spmd(nc, in_maps, core_ids=list(range(NCORES)))
    outs = [res.results[c]["out"].reshape(M, 1) for c in range(NCORES)]
    return np.concatenate(outs, axis=0).astype(np.float32)


# revision 57
# speedup vs baseline: 1.0804x; 1.0804x over previous
"""CPI-MPNN (molecule MPNN + protein CNN + FC head) Trainium2 kernel.

Self-contained: hardcodes all shapes. Shards the batch (128) across 8
NeuronCores (16 samples each), replicates the small weights.

Strategy (v4 = v3 + fp8 DoubleRow conv tower):
  - Protein conv tower in fp8e4 with DoubleRow matmuls: two conv taps
    share one matmul (virtual K=256). The moving operand's tap pair is
    an overlapping 3D access pattern (pair stride = 1 element), so no
    activation duplication is needed. Cuts conv PE cycles ~40%.
  - conv0 input host-packed as channel-pair-interleaved fp8 with all 3
    taps stacked on 75 partitions: one DoubleRow matmul per 500-col
    chunk.
  - Activations carry power-of-2 fp8 scales (x*256); the scales fold
    into the activation scale/bias constants and the fc0 protein rows.
  - MPNN stays bf16 (it dominates the output numerically).
  - Single-matmul interleave of conv (N=500) and MPNN (small) matmuls
    keeps the PE duty cycle above the HAM clock-gate threshold.
  - PE warm-up matmuls during the initial DMA wait.
  - DMA descriptors batched into a few large blobs.
"""

import os
import numpy as np
from contextlib import ExitStack

import concourse.bass as bass
import concourse.tile as tile
from concourse import bacc, mybir
from concourse.bass_utils import run_bass_kernel_spmd



F32 = mybir.dt.float32
BF16 = mybir.dt.bfloat16
FP8 = mybir.dt.float8e4
AF = mybir.ActivationFunctionType
ALU = mybir.AluOpType
DRSW = mybir.MatmulPerfMode.DoubleRowSwInterleave

H = 200
B, NA, NB = 128, 48, 96
L = 1000

NCORES = 8
M = B // NCORES          # samples per core (16)
SEG = 1006               # 3 + 1000 + 3 padded segment
PAD = 3
NCH = 500                # conv free-dim chunk (2 per sample)
GM = 4                   # molecules per DMA group

# fp8 scaling: activations and weights stored *256; every conv layer's
# psum is then true*65536, un-scaled by the next activation's
# scale=2^-8 (and the pool bias / fc0 protein rows for the last layer).
ACT_SC = 1.0 / 256.0
POOL_SC = 65536.0

# weight blob column layouts (all DoubleRow stationaries padded to the
# full 128 columns — the PE rejects DR matmuls with num_active_cols<128)
EBF = 776                # bf16 early blob: wi | fbc group 0 (per-core)
EF8 = 896                # fp8 early blob: w0i | w1p0 | w1p1 | w1s
WBLOB = 1792             # fp8 conv2 blob: w2pA*3 | w2sA | w2pB*3 | w2sB
BLOB1 = 400              # bf16: wha | whb
BLOB2B = 601             # bf16: wo1 | wo2 | wo3 | ones
BLOB3 = 1001             # bf16: fc0 a-d | fc1 a/b | fc2

_CACHE = {}


def _dr_rhs(ap2d, n):
    """[P, 2, n] moving-operand AP with pair stride 2: the two virtual
    rows read taps t and t+2. Pair stride 1 (overlapping 16-bit lane
    reads) hard-crashes the PE when other engines touch SBUF."""
    return bass.AP(ap2d.tensor, ap2d.offset,
                   [list(ap2d.ap[0]), [2, 2], [1, n]])


def _build_nc():
    nc = bacc.Bacc("TRN2", target_bir_lowering=False, debug=False)

    d_x0 = nc.dram_tensor("x0p", [76, M, 2 * SEG], FP8, kind="ExternalInput")
    d_fb = nc.dram_tensor("fbc", [50, M, 144], BF16, kind="ExternalInput")
    d_ab = nc.dram_tensor("aba", [96, M, 144], BF16, kind="ExternalInput")
    d_ebf = nc.dram_tensor("earlybf", [50, EBF], BF16, kind="ExternalInput")
    d_ef8 = nc.dram_tensor("earlyf8", [97, EF8], FP8, kind="ExternalInput")
    d_wf8 = nc.dram_tensor("wf8", [128, WBLOB], FP8, kind="ExternalInput")
    d_w1b = nc.dram_tensor("wblob1", [128, BLOB1], BF16, kind="ExternalInput")
    d_w2bb = nc.dram_tensor("wblob2b", [128, BLOB2B], BF16,
                            kind="ExternalInput")
    d_w3b = nc.dram_tensor("wblob3", [128, BLOB3], BF16, kind="ExternalInput")
    d_bias = nc.dram_tensor("biases", [128, 8], F32, kind="ExternalInput")
    d_out = nc.dram_tensor("out", [1, M], F32, kind="ExternalOutput")

    with tile.TileContext(nc) as tc, ExitStack() as ctx:
        cst = ctx.enter_context(tc.tile_pool(name="cst", bufs=1))
        sbs = ctx.enter_context(tc.tile_pool(name="sbs", bufs=1))
        tmp = ctx.enter_context(tc.tile_pool(name="tmp", bufs=1))
        xp = ctx.enter_context(tc.tile_pool(name="xp", bufs=1))
        pp = ctx.enter_context(tc.tile_pool(name="pp", bufs=1, space="PSUM"))

        # ---- PE warm-up (HAM clock ramp during the DMA wait) ----
        # memset on the vector engine: ~0.3us vs ~0.7us on gpsimd, and
        # DVE's queue comes up early, so the PE starts sooner.
        warm = cst.tile([128, 628], BF16, tag="warm")
        nc.vector.memset(warm[:], 0.0)
        warm_ps = pp.tile([128, NCH], F32, tag="cv", bufs=3, name="warm_ps")

        def emit_warm():
            nc.tensor.matmul(warm_ps[:], warm[:, 0:128], warm[:, 128:628],
                             start=True, stop=True)

        for _ in range(3):
            emit_warm()

        # ---- input DMAs (batched descriptors) ----
        x0_bufs = {}

        def x0_dma(s0, n, eng):
            t = xp.tile([76, n * 2 * SEG], FP8, tag=f"x0s{s0}",
                        name=f"x0s{s0}")
            eng.dma_start(t[:].rearrange("p (s c) -> p s c", c=2 * SEG),
                          d_x0.ap()[:, s0:s0 + n, :])
            for i in range(n):
                x0_bufs[s0 + i] = (t, i * 2 * SEG)

        def x0_stream(s):
            # called from gen_conv0(s): fetch the pair two samples ahead
            nx = s + 2
            if nx % 2 == 0 and nx < M and nx not in x0_bufs:
                eng = nc.sync if nx == 2 else (
                    nc.scalar if (nx // 2) % 2 == 0 else nc.sync)
                x0_dma(nx, 2, eng)

        fb_g, ab_g = {}, {}

        def grp_dma(g, eng, fb_too=True):
            t = cst.tile([96, GM * 144], BF16, tag=f"ab{g}")
            eng.dma_start(t[:].rearrange("p (m i) -> p m i", i=144),
                          d_ab.ap()[:, GM * g:GM * (g + 1), :])
            ab_g[g] = t
            if fb_too:
                t = cst.tile([50, GM * 144], BF16, tag=f"fb{g}")
                eng.dma_start(t[:].rearrange("p (m i) -> p m i", i=144),
                              d_fb.ap()[:, GM * g:GM * (g + 1), :])
                fb_g[g] = t

        # ACT queue (its DMA queue comes up several us before sync's):
        # everything the conv stream needs, in need order, plus the
        # mol stream's critical first inputs packed into earlybf.
        ebf = cst.tile([50, EBF], BF16, tag="ebf")
        nc.scalar.dma_start(ebf[:], d_ebf.ap())
        fb_g[0] = ebf[0:50, 200:EBF]
        ef8 = cst.tile([97, EF8], FP8, tag="ef8")
        nc.scalar.dma_start(ef8[:], d_ef8.ap())
        x0_dma(0, 1, nc.scalar)
        wf8 = cst.tile([128, WBLOB], FP8, tag="wf8")
        nc.scalar.dma_start(wf8[:], d_wf8.ap())
        x0_dma(1, 1, nc.scalar)
        # SP queue: the rest in landing order.
        grp_dma(0, nc.sync, fb_too=False)
        biases = cst.tile([128, 8], F32, tag="biases")
        nc.sync.dma_start(biases[:], d_bias.ap())
        blob1 = cst.tile([128, BLOB1], BF16, tag="blob1")
        nc.sync.dma_start(blob1[:], d_w1b.ap())
        blob2b = cst.tile([128, BLOB2B], BF16, tag="blob2b")
        nc.sync.dma_start(blob2b[:], d_w2bb.ap())
        grp_dma(1, nc.sync)
        grp_dma(2, nc.sync)
        grp_dma(3, nc.sync)
        blob3 = cst.tile([128, BLOB3], BF16, tag="blob3")
        nc.sync.dma_start(blob3[:], d_w3b.ap())

        # fp8 conv weight views: SW-interleaved pair blocks (contiguous
        # fast weight load; plain DoubleRow's HW interleave pays ~150ns
        # extra LDWEIGHTS whenever a short matmul precedes it). conv0/
        # conv1 carry their bias as an extra contraction row (b*512
        # against a constant-128 activation row) so the relu runs as a
        # two-op tensor_scalar on the otherwise-idle gpsimd engine.
        w0i_t = ef8[0:76, 0:256]
        w1p_t = [ef8[0:96, 256 + 256 * j:512 + 256 * j] for j in range(2)]
        w1s_t = ef8[0:97, 768:896]
        w2pA_t = [wf8[0:128, 256 * j:256 + 256 * j] for j in range(3)]
        w2sA_t = wf8[0:128, 768:896]
        w2pB_t = [wf8[0:128, 896 + 256 * j:1152 + 256 * j] for j in range(3)]
        w2sB_t = wf8[0:128, 1664:1792]

        # bf16 weight views
        wi_t = ebf[0:50, 0:200]
        wha_t = blob1[0:128, 0:200]
        whb_t = blob1[0:72, 200:400]
        wo1_t = blob2b[0:40, 0:200]
        wo2_t = blob2b[0:128, 200:400]
        wo3_t = blob2b[0:72, 400:600]
        fc0_t = [blob3[0:128, 0:200], blob3[0:72, 200:400],
                 blob3[0:128, 400:600], blob3[0:72, 600:800]]
        fc1a_t = blob3[0:128, 800:900]
        fc1b_t = blob3[0:72, 900:1000]
        fc2w_t = blob3[0:100, 1000:1001]
        b0_t = biases[0:96, 0:1]
        b1_t = biases[0:128, 1:2]
        b2a_t = biases[0:128, 2:3]
        b2b_t = biases[0:72, 3:4]
        fc0ba_t = biases[0:128, 4:5]
        fc0bb_t = biases[0:72, 5:6]
        fc1bias_t = biases[0:100, 6:7]
        fc2b_t = biases[0:1, 7:8]

        # static outputs of the two towers, feature-major [feat, M]
        embT1 = sbs.tile([128, M], BF16, tag="embT1")
        embT2 = sbs.tile([72, M], BF16, tag="embT2")
        embT1f = sbs.tile([128, M], F32, tag="embT1f")
        embT2f = sbs.tile([72, M], F32, tag="embT2f")
        prT1p = sbs.tile([128, M], F32, tag="prT1p")
        prT2p = sbs.tile([72, M], F32, tag="prT2p")

        # ================= per-molecule MPNN (staged generators) =======
        mol_state = {}

        def gen_binput(m):
            g, r = m // GM, m % GM
            fb_m = fb_g[g][:, r * 144:r * 144 + 96]
            ps = pp.tile([96, 200], F32, tag="mp", bufs=3)
            nc.tensor.matmul(ps[:], fb_m, wi_t, start=True, stop=True)
            binp = sbs.tile([96, 200], F32, tag=f"binp{m}")
            nc.vector.tensor_copy(binp[:], ps[:])
            msg = sbs.tile([96, 200], BF16, tag=f"msg{m}")
            nc.scalar.activation(msg[:], ps[:], AF.Relu)
            mol_state[m] = (binp, msg)
            yield

        def gen_iter_pre(m):
            g, r = m // GM, m % GM
            ab_m = ab_g[g][:, r * 144:r * 144 + 96]
            binp, msg = mol_state[m]
            # pa/pb share one PSUM bank slot; both are single-matmul
            # accumulation groups so the bank-granular pending-zero mark
            # of the second can't corrupt the first mid-group.
            nt = pp.tile([128, 192], F32, tag="nt", bufs=2)
            nc.tensor.matmul(nt[0:128, 0:96], msg[:, 0:128], ab_m,
                             start=True, stop=True)
            yield
            nc.tensor.matmul(nt[0:72, 96:192], msg[:, 128:200], ab_m,
                             start=True, stop=True)
            nTa = tmp.tile([128, 96], BF16, tag="nTa", bufs=6)
            nc.vector.tensor_copy(nTa[:], nt[0:128, 0:96])
            nTb = tmp.tile([72, 96], BF16, tag="nTb", bufs=6)
            nc.vector.tensor_copy(nTb[:], nt[0:72, 96:192])
            mol_state[m] = (binp, msg, nTa, nTb)
            yield

        def gen_iter_post(m):
            binp, msg, nTa, nTb = mol_state[m]
            ps = pp.tile([96, 200], F32, tag="mp", bufs=3)
            nc.tensor.matmul(ps[:], nTa[:], wha_t, start=True, stop=False)
            yield
            nc.tensor.matmul(ps[:], nTb[:], whb_t, start=False, stop=True)
            tm = tmp.tile([96, 200], F32, tag="mtmp", bufs=3)
            nc.vector.tensor_add(tm[:], ps[:], binp[:])
            nc.scalar.activation(msg[:], tm[:], AF.Relu)
            mol_state[m] = (binp, msg)
            yield

        atom_g = {}

        def gen_atom_pre(m):
            g, r = m // GM, m % GM
            aa_m = ab_g[g][:, r * 144 + 96:(r + 1) * 144]
            binp, msg = mol_state[m]
            if r == 0:
                atom_g[g] = (tmp.tile([128, 192], BF16, tag="nat1", bufs=2,
                                      name="nat1g"),
                             tmp.tile([72, 192], BF16, tag="nat2", bufs=2,
                                      name="nat2g"))
            nat1_g, nat2_g = atom_g[g]
            pT = pp.tile([128, 96], F32, tag="nt", bufs=2)
            nc.tensor.matmul(pT[0:128, 0:48], msg[:, 0:128], aa_m,
                             start=True, stop=True)
            yield
            nc.tensor.matmul(pT[0:72, 48:96], msg[:, 128:200], aa_m,
                             start=True, stop=True)
            nc.scalar.copy(nat1_g[:, r * 48:(r + 1) * 48], pT[0:128, 0:48])
            nc.scalar.copy(nat2_g[:, r * 48:(r + 1) * 48], pT[0:72, 48:96])
            yield

        def gen_atom_mm(g):
            # batched over the 4 molecules of the group: one stationary
            # load per W_o block, moving operand N=192. Atom hidden
            # states come out H-major so the atom mean collapses into
            # the activation's accum_out; W_o pre-scaled 1/48 on host.
            nat1_g, nat2_g = atom_g[g]
            fb0 = fb_g[g][0:40, 96:97]
            c1_g = bass.AP(fb0.tensor, fb0.offset,
                           [list(fb0.ap[0]), [144, GM], [1, 48]])
            for half, wid, embf in ((0, 128, embT1f), (1, 72, embT2f)):
                lo, hi = (0, 128) if half == 0 else (128, 200)
                ps = pp.tile([wid, 192], F32, tag="nt", bufs=2,
                             name=f"psAH{half}")
                nc.tensor.matmul(ps[:], wo1_t[:, lo:hi], c1_g,
                                 start=True, stop=False)
                yield
                nc.tensor.matmul(ps[:], wo2_t[:, lo:hi], nat1_g[:],
                                 start=False, stop=False)
                yield
                nc.tensor.matmul(ps[:], wo3_t[:, lo:hi], nat2_g[:],
                                 start=False, stop=True)
                for r in range(GM):
                    rh = tmp.tile([wid, 48], BF16, tag="reluh", bufs=4,
                                  name=f"r{half}")
                    nc.scalar.activation(rh[:], ps[:, r * 48:(r + 1) * 48],
                                         AF.Relu,
                                         accum_out=embf[:, GM * g + r:
                                                        GM * g + r + 1])
                yield

        # ================= per-sample protein conv tower (fp8) =========
        sample_state = {}

        def gen_conv0(s):
            x0t, off = x0_bufs[s]
            x1 = xp.tile([97, SEG], FP8, tag="x1", bufs=3)
            nc.gpsimd.memset(x1[0:96, 0:PAD], 0.0)
            nc.gpsimd.memset(x1[0:96, PAD + 1000:SEG], 0.0)
            nc.gpsimd.memset(x1[96:97, :], 128.0)   # conv1 bias-row input
            for c in range(2):
                base = off + PAD + c * NCH
                ps = pp.tile([128, NCH], F32, tag="cv", bufs=3)
                # host layout: [76, 2, SEG] = 3 taps x 25 ch-pairs (+ a
                # constant bias row), pre-shifted per tap; one DoubleRow
                # matmul per chunk.
                rhs = bass.AP(x0t[:].tensor, x0t[:, base:base + 1].offset,
                              [list(x0t[:].ap[0]), [SEG, 2], [1, NCH]])
                nc.tensor.matmul(ps[:], w0i_t, rhs, start=True, stop=True,
                                 perf_mode=DRSW)
                yield
                nc.scalar.activation(x1[0:96, PAD + c * NCH:
                                        PAD + (c + 1) * NCH],
                                     ps[0:96, :], AF.Relu, scale=ACT_SC)
                yield
            x0_stream(s)
            sample_state[s] = [x1, None, None, None]

        # chunk splits chosen so chunk 0 of each layer reads only chunk
        # 0 of its producer (the conv window would otherwise reach 2-3
        # columns past the boundary and serialize on BOTH activations).
        C1SPL = (0, 498, 1000)
        C2SPL = (0, 495, 1000)

        def gen_conv1(s, c):
            st = sample_state[s]
            x1 = st[0]
            if c == 0:
                x2 = xp.tile([128, SEG], FP8, tag="x2", bufs=4)
                nc.gpsimd.memset(x2[:, 0:PAD], 0.0)
                nc.gpsimd.memset(x2[:, PAD + 1000:SEG], 0.0)
                st[1] = x2
            x2 = st[1]
            base = PAD + C1SPL[c]
            n = C1SPL[c + 1] - C1SPL[c]
            ps = pp.tile([128, n], F32, tag="cv", bufs=3, name="c1ps")
            for j, t in enumerate((0, 1)):       # pairs (0,2) and (1,3)
                rhs = _dr_rhs(x1[0:96, base + t - 2:base + t + n], n)
                nc.tensor.matmul(ps[:], w1p_t[j], rhs, start=(j == 0),
                                 stop=False, perf_mode=DRSW)
                yield
            nc.tensor.matmul(ps[:], w1s_t, x1[0:97, base + 2:base + 2 + n],
                             start=False, stop=True)
            nc.scalar.activation(x2[:, base:base + n], ps[:],
                                 AF.Relu, scale=ACT_SC)
            yield

        def gen_conv2(s, c, half):
            st = sample_state[s]
            x2 = st[1]
            if half == 0:
                if c == 0:
                    st[2] = tmp.tile([128, 2], F32, tag="mxA", bufs=3,
                                     name="mxA")
                    st[3] = tmp.tile([72, 2], F32, tag="mxB", bufs=3,
                                     name="mxB")
                wp, ws, use, mx = w2pA_t, w2sA_t, 128, st[2]
            else:
                wp, ws, use, mx = w2pB_t, w2sB_t, 72, st[3]
            base = PAD + C2SPL[c]
            n = C2SPL[c + 1] - C2SPL[c]
            ps = pp.tile([128, n], F32, tag="cv", bufs=3, name="c2ps")
            for j, t in enumerate((0, 4, 1)):    # pairs (0,2) (4,6) (1,3)
                rhs = _dr_rhs(x2[:, base + t - 3:base + t - 1 + n], n)
                nc.tensor.matmul(ps[:], wp[j], rhs, start=(j == 0),
                                 stop=False, perf_mode=DRSW)
                yield
            nc.tensor.matmul(ps[:], ws, x2[:, base + 2:base + 2 + n],
                             start=False, stop=True)
            nc.vector.reduce_max(mx[:, c:c + 1], ps[0:use, :],
                                 axis=mybir.AxisListType.X)
            if c == 1:
                dst = prT1p if half == 0 else prT2p
                nc.vector.reduce_max(dst[:, s:s + 1], mx[:],
                                     axis=mybir.AxisListType.X)
                if half == 1:
                    sample_state.pop(s)
            yield

        # FC first-layer matmuls on the molecule embeddings run woven
        # into the last conv pieces; psums allocated lazily to preserve
        # the nt slot rotation.
        fc_ps = {}

        def gen_fc_early():
            nc.gpsimd.tensor_copy(embT1[:], embT1f[:])
            nc.gpsimd.tensor_copy(embT2[:], embT2f[:])
            fc_ps["a"] = pp.tile([128, M], F32, tag="nt", bufs=2, name="ps0a")
            fc_ps["b"] = pp.tile([72, M], F32, tag="nt", bufs=2, name="ps0b")
            rhs2 = (embT1, embT2)
            for k in range(2):
                nc.tensor.matmul(fc_ps["a"][:], fc0_t[k][:, 0:128],
                                 rhs2[k][:], start=(k == 0), stop=False)
                yield
            for k in range(2):
                nc.tensor.matmul(fc_ps["b"][:], fc0_t[k][:, 128:200],
                                 rhs2[k][:], start=(k == 0), stop=False)
                yield

        # ---- build the two global piece streams and weave them ----
        conv_pieces = []
        mol_pieces = []
        for g in range(4):
            s0, s1, s2, s3 = (4 * g + i for i in range(4))
            mols = [GM * g + r for r in range(GM)]
            for stage in (gen_binput, gen_iter_pre, gen_iter_post,
                          gen_iter_pre, gen_iter_post, gen_atom_pre):
                for m in mols:
                    mol_pieces.append(stage(m))
            mol_pieces.append(gen_atom_mm(g))
            conv_pieces += [
                gen_conv0(s0), gen_conv0(s1),
                gen_conv1(s0, 0), gen_conv1(s1, 0),
                gen_conv1(s0, 1), gen_conv1(s1, 1),
                gen_conv2(s0, 0, 0), gen_conv2(s0, 0, 1),
                gen_conv2(s1, 0, 0), gen_conv2(s1, 0, 1),
                gen_conv0(s2), gen_conv1(s2, 0),
                gen_conv2(s0, 1, 0), gen_conv2(s0, 1, 1),
                gen_conv1(s2, 1),
                gen_conv2(s1, 1, 0), gen_conv2(s1, 1, 1),
                gen_conv0(s3), gen_conv1(s3, 0), gen_conv1(s3, 1),
                gen_conv2(s2, 0, 0), gen_conv2(s2, 0, 1),
                gen_conv2(s2, 1, 0), gen_conv2(s2, 1, 1),
                gen_conv2(s3, 0, 0), gen_conv2(s3, 0, 1),
                gen_conv2(s3, 1, 0), gen_conv2(s3, 1, 1),
            ]
        mol_pieces.append(gen_fc_early())

        def stream(pieces):
            for p in pieces:
                yield from p

        cs = stream(conv_pieces)
        ms = stream(mol_pieces)
        # lead with the first mol group's binput+iter_pre; warmup
        # matmuls fill this phase's dependency stalls
        for _ in range(12):
            next(ms, None)
            emit_warm()
        conv_alive = mol_alive = True
        credit = 8.0
        RATIO = 104.0 / 51.0
        while conv_alive or mol_alive:
            credit += RATIO
            while credit >= 1.0 and conv_alive:
                credit -= 1.0
                if next(cs, StopIteration) is StopIteration:
                    conv_alive = False
            if mol_alive and next(ms, StopIteration) is StopIteration:
                mol_alive = False
            if not conv_alive:
                credit = 0.0

        # maxpool -> bias -> relu (monotone, so pool-first is exact);
        # prT carries the 65536x conv2 scale, folded into fc0 rows.
        prT1 = sbs.tile([128, M], BF16, tag="prT1")
        nc.vector.tensor_scalar(prT1[:], prT1p[:], b2a_t, 0.0,
                                op0=ALU.add, op1=ALU.max)
        prT2 = sbs.tile([72, M], BF16, tag="prT2")
        nc.vector.tensor_scalar(prT2[:], prT2p[:], b2b_t, 0.0,
                                op0=ALU.add, op1=ALU.max)

        # ================= FC head (tail) =================
        rhs4 = (embT1, embT2, prT1, prT2)
        for k in (2, 3):
            nc.tensor.matmul(fc_ps["a"][:], fc0_t[k][:, 0:128], rhs4[k][:],
                             start=False, stop=(k == 3))
        h0a = tmp.tile([128, M], BF16, tag="h0a")
        nc.scalar.activation(h0a[:], fc_ps["a"][:], AF.Relu, bias=fc0ba_t)
        for k in (2, 3):
            nc.tensor.matmul(fc_ps["b"][:], fc0_t[k][:, 128:200], rhs4[k][:],
                             start=False, stop=(k == 3))
        h0b = tmp.tile([72, M], BF16, tag="h0b")
        nc.scalar.activation(h0b[:], fc_ps["b"][:], AF.Relu, bias=fc0bb_t)

        ps1 = pp.tile([100, M], F32, tag="nt", bufs=2)
        nc.tensor.matmul(ps1[:], fc1a_t, h0a[:], start=True, stop=False)
        nc.tensor.matmul(ps1[:], fc1b_t, h0b[:], start=False, stop=True)
        h1 = tmp.tile([100, M], BF16, tag="h1")
        nc.scalar.activation(h1[:], ps1[:], AF.Relu, bias=fc1bias_t)

        ps2 = pp.tile([1, M], F32, tag="nt", bufs=2)
        nc.tensor.matmul(ps2[:], fc2w_t, h1[:], start=True, stop=True)
        outsb = tmp.tile([1, M], F32, tag="outsb")
        nc.scalar.add(outsb[:], ps2[:], fc2b_t[0:1, 0:1])
        nc.sync.dma_start(d_out.ap(), outsb[:])

    nc.compile()
    return nc


def _prep(inputs):
    """Host preprocessing: returns the 8 per-core in_maps."""
    import ml_dtypes
    f32 = np.float32
    bf16 = ml_dtypes.bfloat16
    fp8 = ml_dtypes.float8_e4m3
    SC = 256.0
    fatoms = np.asarray(inputs["fatoms"], f32)
    fbonds = np.asarray(inputs["fbonds"], f32)
    agraph = np.asarray(inputs["agraph"])
    bgraph = np.asarray(inputs["bgraph"])
    pseq = np.asarray(inputs["protein_seq"])
    W_i = np.asarray(inputs["W_i"], f32)
    W_h = np.asarray(inputs["W_h"], f32)
    W_o_w = np.asarray(inputs["W_o_w"], f32)
    W_o_b = np.asarray(inputs["W_o_b"], f32)
    embp = np.asarray(inputs["embed_protein"], f32)

    # protein embeddings *256, channel-pair interleaved with the 3 taps
    # pre-shifted and stacked on 75 partitions (+ the conv0 bias input
    # row, constant 128): [76, 2, SEG] per sample
    pvT = np.ascontiguousarray(embp[pseq].transpose(0, 2, 1)) * SC  # (B,50,L)
    x0i = np.zeros((B, 76, 2, SEG), f32)
    for tap in range(3):
        sh = tap - 1                      # position shift: pv[ch, c-PAD+sh]
        lo, hi = max(0, PAD - sh), min(SEG, PAD + L - sh)
        blk = x0i[:, tap * 25:(tap + 1) * 25]
        blk[:, :, 0, lo:hi] = pvT[:, 0::2, lo - PAD + sh:hi - PAD + sh]
        blk[:, :, 1, lo:hi] = pvT[:, 1::2, lo - PAD + sh:hi - PAD + sh]
    x0i[:, 75, 0, :] = 128.0
    x0i = x0i.reshape(B, 76, 2 * SEG).astype(fp8)

    # adjacency one-hots (counts; contraction-dim-major)
    ar = np.arange(B)[:, None, None]
    cntB = np.zeros((B, NB, NB), f32)
    np.add.at(cntB, (ar, np.arange(NB)[None, :, None], bgraph), 1.0)
    abt = cntB.transpose(0, 2, 1)                              # (B, j, i)
    cntA = np.zeros((B, NA, NB), f32)
    np.add.at(cntA, (ar, np.arange(NA)[None, :, None], agraph), 1.0)
    aat = cntA.transpose(0, 2, 1)                              # (B, j, a)

    # combined group inputs: ab = [abt | aat] on 96 partitions,
    # fb = [fbonds.T | cat1 (40 rows + 10 pad)] on 50 partitions
    aba = np.concatenate([abt, aat], axis=2)                   # (B, 96, 144)
    fbT = fbonds.transpose(0, 2, 1)                            # (B, 50, 96)
    faT = fatoms.transpose(0, 2, 1)                            # (B, 39, 48)
    cat1 = np.concatenate(
        [faT, np.ones((B, 1, NA), f32), np.zeros((B, 10, NA), f32)], axis=1)
    fbc = np.concatenate([fbT, cat1], axis=2)                  # (B, 50, 144)

    conv_w = [np.asarray(inputs[f"conv{i}_w"], f32) for i in range(3)]
    conv_b = [np.asarray(inputs[f"conv{i}_b"], f32) for i in range(3)]
    fcw = [np.asarray(inputs[f"fc{i}_w"], f32) for i in range(3)]
    fcb = [np.asarray(inputs[f"fc{i}_b"], f32) for i in range(3)]

    def put(blob, col, rows, a):
        blob[0:rows, col:col + a.shape[1]] = a
        return col + a.shape[1]

    def sw_pair(dst, rows, col, wa, wb):
        """Store a DoubleRowSwInterleave pair block: viewed col 2k =
        A[:, 127-k], col 2k+1 = B[:, 127-k]; A/B zero-padded to the
        mandatory 128 columns."""
        a = np.zeros((rows, 128), f32)
        a[:, 0:wa.shape[1]] = wa
        b = np.zeros((rows, 128), f32)
        b[:, 0:wb.shape[1]] = wb
        dst[0:rows, col:col + 256:2] = a[:, ::-1]
        dst[0:rows, col + 1:col + 256:2] = b[:, ::-1]

    # fp8 early blob: w0i | w1p0 | w1p1 | w1s  (all *256; tap pairs are
    # (t, t+2) — the DR moving operand uses pair stride 2). The conv
    # biases ride as extra contraction rows: weight b*512 against the
    # constant-128 activation row gives b*65536 in the psum.
    ef8 = np.zeros((97, EF8), f32)
    w0 = conv_w[0] * SC                                        # (96, 50, 3)
    w0a = np.zeros((76, 96), f32)
    w0b = np.zeros((76, 96), f32)
    for tap in range(3):
        w0a[tap * 25:(tap + 1) * 25] = w0[:, 0::2, tap].T
        w0b[tap * 25:(tap + 1) * 25] = w0[:, 1::2, tap].T
    w0a[75] = conv_b[0] * 512.0
    sw_pair(ef8, 76, 0, w0a, w0b)
    w1 = conv_w[1] * SC                                        # (128, 96, 5)
    for j, t in enumerate((0, 1)):
        sw_pair(ef8, 96, 256 + 256 * j, w1[:, :, t].T, w1[:, :, t + 2].T)
    ef8[0:96, 768:896] = w1[:, :, 4].T
    ef8[96, 768:896] = conv_b[1] * 512.0
    # fp8 conv2 blob: w2pA*3 | w2sA | w2pB*3 | w2sB
    wf8 = np.zeros((128, WBLOB), f32)
    w2 = conv_w[2] * SC                                        # (200, 128, 7)
    for j, t in enumerate((0, 4, 1)):
        sw_pair(wf8, 128, 256 * j, w2[0:128, :, t].T, w2[0:128, :, t + 2].T)
        sw_pair(wf8, 128, 896 + 256 * j,
                w2[128:200, :, t].T, w2[128:200, :, t + 2].T)
    wf8[0:128, 768:896] = w2[0:128, :, 5].T
    wf8[0:128, 1664:1736] = w2[128:200, :, 5].T

    # blob1: wha | whb  (bf16 MPNN weights)
    blob1 = np.zeros((128, BLOB1), f32)
    c = put(blob1, 0, 128, W_h[0:128])
    c = put(blob1, c, 72, W_h[128:200])
    assert c == BLOB1

    # blob2b: wo1 | wo2 | wo3 | ones
    wo1 = np.zeros((40, 200), f32)
    wo1[:39] = W_o_w[0:39] / 48.0
    wo1[39] = W_o_b / 48.0
    blob2b = np.zeros((128, BLOB2B), f32)
    c = put(blob2b, 0, 40, wo1)
    c = put(blob2b, c, 128, W_o_w[39:167] / 48.0)
    c = put(blob2b, c, 72, W_o_w[167:239] / 48.0)
    c = put(blob2b, c, 48, np.ones((48, 1), f32))
    assert c == BLOB2B

    # blob3: fc0 a-d | fc1 a/b | fc2; the fc0 protein rows (200:400)
    # un-scale the 65536x pooled conv2 output.
    fc0 = fcw[0].copy()
    fc0[200:400] /= POOL_SC
    blob3 = np.zeros((128, BLOB3), f32)
    c = put(blob3, 0, 128, fc0[0:128])
    c = put(blob3, c, 72, fc0[128:200])
    c = put(blob3, c, 128, fc0[200:328])
    c = put(blob3, c, 72, fc0[328:400])
    c = put(blob3, c, 128, fcw[1][0:128])
    c = put(blob3, c, 72, fcw[1][128:200])
    c = put(blob3, c, 100, fcw[2])
    assert c == BLOB3

    bias = np.zeros((128, 8), f32)
    bias[0:96, 0] = conv_b[0] * SC
    bias[0:128, 1] = conv_b[1] * SC
    bias[0:128, 2] = conv_b[2][0:128] * POOL_SC
    bias[0:72, 3] = conv_b[2][128:200] * POOL_SC
    bias[0:128, 4] = fcb[0][0:128]
    bias[0:72, 5] = fcb[0][128:200]
    bias[0:100, 6] = fcb[1]
    bias[0:1, 7] = fcb[2]

    shared = {
        "earlyf8": ef8.astype(fp8),
        "wf8": wf8.astype(fp8),
        "wblob1": blob1.astype(bf16),
        "wblob2b": blob2b.astype(bf16),
        "wblob3": blob3.astype(bf16), "biases": bias,
    }

    x0_cm = x0i.transpose(1, 0, 2)                             # (75, B, 2SEG)
    fbc_cm = fbc.transpose(1, 0, 2)                            # (50, B, 144)
    aba_cm = aba.transpose(1, 0, 2)                            # (96, B, 144)

    in_maps = []
    for cix in range(NCORES):
        lo = cix * M
        im = dict(shared)
        im["x0p"] = np.ascontiguousarray(x0_cm[:, lo:lo + M, :])
        im["fbc"] = np.ascontiguousarray(fbc_cm[:, lo:lo + M, :]).astype(bf16)
        im["aba"] = np.ascontiguousarray(aba_cm[:, lo:lo + M, :]).astype(bf16)
        # earlybf: W_i | this core's group-0 fbc (molecules 0-3)
        ebf = np.zeros((50, EBF), f32)
        ebf[:, 0:200] = W_i
        ebf[:, 200:EBF] = fbc_cm[:, lo:lo + GM, :].reshape(50, GM * 144)
        im["earlybf"] = ebf.astype(bf16)
        in_maps.append(im)
    return in_maps


def get_nc():
    if "nc" not in _CACHE:
        _CACHE["nc"] = _build_nc()
    return _CACHE["nc"]


def kernel(**inputs) -> np.ndarray:
    nc = get_nc()
    in_maps = _prep(inputs)
    res = run_bass_kernel_spmd(nc, in_maps, core_ids=list(range(NCORES)))
    outs = [res.results[c]["out"].reshape(M, 1) for c in range(NCORES)]
    return np.concatenate(outs, axis=0).astype(np.float32)
